# revision 8
# baseline (speedup 1.0000x reference)
"""Trainium2 Bass kernel: embedding gather + Conv1d(k=5,pad=2) + positional add.

Reference computation (shapes hardcoded):
  X:        (8, 8192) int64 token ids in [0, 100000)
  W_lin:    (128, 100000) f32   emb = W_lin.T[X] + b_lin          -> (8, 8192, 128)
  W_conv:   (128, 128, 5) f32   y = conv1d(emb.T, W_conv, pad=2) + b_conv
  pos_table:(8192, 128) f32     out = y.T + pos_table[:8192]      -> (8, 8192, 128)

Sharding: sequence-parallel across 8 NeuronCores. Core c computes output
tokens [c*1024, (c+1)*1024) for all 8 batch rows. The linear bias b_lin,
conv bias b_conv and pos_table are folded into a single per-position bias
table on the host (conv is linear, so conv(emb + b_lin) = conv(emb) +
edge-aware-const(b_lin)).

Device pipeline per core, per batch row:
  1. indirect-DMA gather of 9*128 = 1152 embedding rows (2 halo tokens per
     side, padded with a zero row at table index 100000) -> SBUF, token on
     partition, dim on free axis.
  2. PE transposes (via identity matmul) -> embT [dim=128 part, 1152 tokens]
  3. conv as 5 accumulating matmuls per 512-token tile:
     psum[dout, n] += W_k[din,dout]^T @ embT[din, n+k],  float32r full-rate
  4. PE transposes of the conv output back to [token, dout]
  5. DVE add of the folded bias table, DMA out.
"""

import os
import sys

sys.path.insert(0, "/opt/trn_rl_repo")

import numpy as np

VOCAB = 100000
MAX_SEQ = 8192
DIM = 128
KW = 5
PAD = 2
B = 8
NCORES = 8
CHUNK = MAX_SEQ // NCORES          # 1024 output tokens per core
NBLK = 9                           # gathered 128-token blocks per row (1152 >= 1024 + 4 halo)
GATH = NBLK * 128                  # 1152 gathered positions per row

_CACHE = {}


def _build_nc(iters=1):
    from concourse import bacc, bass, mybir, tile
    from concourse.masks import make_identity

    f32 = mybir.dt.float32
    f32r = mybir.dt.float32r
    i32 = mybir.dt.int32

    nc = bacc.Bacc(None, target_bir_lowering=False)
    table_d = nc.declare_dram_parameter("table", [VOCAB + 1, DIM], f32, isOutput=False)
    idx_d = nc.declare_dram_parameter("idx", [128, B * NBLK], i32, isOutput=False)
    bias_d = nc.declare_dram_parameter("bias", [128, CHUNK], f32, isOutput=False)
    wk_d = nc.declare_dram_parameter("wk", [128, KW * DIM], f32r, isOutput=False)
    out_d = nc.declare_dram_parameter("out", [B, 128, CHUNK // 128, DIM], f32, isOutput=True)

    with tile.TileContext(nc) as tc:
        with (
            tc.tile_pool(name="const", bufs=1) as constp,
            tc.tile_pool(name="g", bufs=3) as gpool,
            tc.tile_pool(name="embT", bufs=3) as epool,
            tc.tile_pool(name="csb", bufs=3) as cspool,
            tc.tile_pool(name="orow", bufs=3) as orowp,
            tc.tile_pool(name="pt", bufs=2, space="PSUM") as tpool,
            tc.tile_pool(name="pc", bufs=2, space="PSUM") as cpool,
            tc.tile_pool(name="po", bufs=2, space="PSUM") as opool,
        ):
            idx_sb = constp.tile([128, B * NBLK], i32)
            nc.sync.dma_start(out=idx_sb[:, :], in_=idx_d[:, :])
            bias_sb = constp.tile([128, CHUNK], f32)
            nc.sync.dma_start(out=bias_sb[:, :], in_=bias_d[:, :])
            wk_sb = constp.tile([128, KW * DIM], f32r)
            nc.sync.dma_start(out=wk_sb[:, :], in_=wk_d[:, :])
            ident = constp.tile([128, 128], f32)
            make_identity(nc, ident[:, :])

            import contextlib

            loop_cm = (
                tc.For_i(0, iters, 1, hint_engines=(mybir.EngineType.PE,))
                if iters > 1
                else contextlib.nullcontext()
            )
            with loop_cm:
                body(nc, tc, bass, mybir, idx_sb, bias_sb, wk_sb, ident,
                     table_d, out_d, gpool, epool, cspool, orowp,
                     tpool, cpool, opool)
    if not nc.is_finalized():
        nc.finalize()
    return nc


def body(nc, tc, bass, mybir, idx_sb, bias_sb, wk_sb, ident, table_d, out_d,
         gpool, epool, cspool, orowp, tpool, cpool, opool):
    f32 = mybir.dt.float32
    f32r = mybir.dt.float32r
    if True:
        if True:
            for b in range(B):
                g = gpool.tile([128, GATH], f32)
                for blk in range(NBLK):
                    col = b * NBLK + blk
                    nc.gpsimd.indirect_dma_start(
                        out=g[:, blk * 128:(blk + 1) * 128],
                        out_offset=None,
                        in_=table_d[:, :],
                        in_offset=bass.IndirectOffsetOnAxis(
                            ap=idx_sb[:, col:col + 1], axis=0
                        ),
                    )
                embT = epool.tile([128, GATH], f32r)
                for grp in range(3):
                    pt = tpool.tile([128, 384], f32)
                    for u in range(3):
                        j = grp * 3 + u
                        nc.tensor.transpose(
                            out=pt[:, u * 128:(u + 1) * 128],
                            in_=g[:, j * 128:(j + 1) * 128],
                            identity=ident[:, :],
                        )
                    nc.scalar.copy(out=embT[:, grp * 384:(grp + 1) * 384], in_=pt[:, :])

                out_row = orowp.tile([128, CHUNK], f32)
                for t in range(2):
                    pc = cpool.tile([128, 512], f32)
                    for k in range(KW):
                        nc.tensor.matmul(
                            out=pc[:, :],
                            lhsT=wk_sb[:, k * DIM:(k + 1) * DIM],
                            rhs=embT[:, t * 512 + k: t * 512 + k + 512],
                            start=(k == 0),
                            stop=(k == KW - 1),
                        )
                    csb = cspool.tile([128, 512], f32)
                    nc.scalar.copy(out=csb[:, :], in_=pc[:, :])
                    po = opool.tile([128, 512], f32)
                    for u in range(4):
                        nc.tensor.transpose(
                            out=po[:, u * 128:(u + 1) * 128],
                            in_=csb[:, u * 128:(u + 1) * 128],
                            identity=ident[:, :],
                        )
                    nc.vector.tensor_add(
                        out_row[:, t * 512:(t + 1) * 512],
                        po[:, :],
                        bias_sb[:, t * 512:(t + 1) * 512],
                    )
                nc.sync.dma_start(
                    out=out_d[b],
                    in_=out_row[:, :].rearrange("p (blk d) -> p blk d", blk=CHUNK // 128),
                )
    return nc


def _prep_inputs(X, W_lin, b_lin, W_conv, b_conv, pos_table):
    """Host-side: table with zero pad row, per-core gather indices and the
    folded bias table (b_lin conv response + b_conv + pos_table)."""
    X = np.asarray(X)
    W_lin = np.asarray(W_lin, dtype=np.float32)
    b_lin = np.asarray(b_lin, dtype=np.float32)
    W_conv = np.asarray(W_conv, dtype=np.float32)
    b_conv = np.asarray(b_conv, dtype=np.float32)
    pos_table = np.asarray(pos_table, dtype=np.float32)

    table = np.empty((VOCAB + 1, DIM), dtype=np.float32)
    table[:VOCAB] = W_lin.T
    table[VOCAB] = 0.0

    # conv response of the constant b_lin, edge-aware (zero padding)
    wb = np.einsum("oik,i->ko", W_conv, b_lin)  # [5, dout]
    conv_lin = np.broadcast_to(wb.sum(0), (MAX_SEQ, DIM)).copy()
    conv_lin[0] = wb[2:].sum(0)
    conv_lin[1] = wb[1:].sum(0)
    conv_lin[MAX_SEQ - 2] = wb[:4].sum(0)
    conv_lin[MAX_SEQ - 1] = wb[:3].sum(0)
    bias_total = conv_lin + b_conv[None, :] + pos_table  # [8192, 128]

    wk_arr = np.ascontiguousarray(
        W_conv.transpose(1, 2, 0).reshape(DIM, KW * DIM)
    )  # [din, k*128+dout]

    Xi = X.astype(np.int64)
    j = np.arange(GATH)
    in_maps = []
    for c in range(NCORES):
        a = c * CHUNK + j - PAD  # absolute gathered positions
        valid = (a >= 0) & (a < MAX_SEQ)
        gi = np.where(valid[None, :], Xi[:, np.clip(a, 0, MAX_SEQ - 1)], VOCAB)
        idx_c = np.ascontiguousarray(
            gi.reshape(B, NBLK, 128).transpose(2, 0, 1).reshape(128, B * NBLK)
        ).astype(np.int32)
        bias_c = np.ascontiguousarray(
            bias_total[c * CHUNK:(c + 1) * CHUNK]
            .reshape(CHUNK // 128, 128, DIM)
            .transpose(1, 0, 2)
            .reshape(128, CHUNK)
        )
        in_maps.append({"table": table, "idx": idx_c, "bias": bias_c, "wk": wk_arr})
    return in_maps


def kernel(X, W_lin, b_lin, W_conv, b_conv, pos_table):
    from concourse.bass_utils import run_bass_kernel_spmd

    iters = int(os.environ.get("KERNEL_ITERS", "1"))
    key = ("nc", iters)
    if key not in _CACHE:
        _CACHE[key] = _build_nc(iters)
    nc = _CACHE[key]

    in_maps = _prep_inputs(X, W_lin, b_lin, W_conv, b_conv, pos_table)
    res = run_bass_kernel_spmd(nc, in_maps, core_ids=list(range(NCORES)))
    _CACHE["last_results"] = res

    full = np.empty((B, MAX_SEQ, DIM), dtype=np.float32)
    for c in range(NCORES):
        o = res.results[c]["out"]  # [B, 128, CHUNK//128, DIM] (b, p, blk, d)
        full[:, c * CHUNK:(c + 1) * CHUNK, :] = (
            o.transpose(0, 2, 1, 3).reshape(B, CHUNK, DIM)
        )
    return full
